# revision 10
# baseline (speedup 1.0000x reference)
"""CapsNet dynamic-routing kernel for Trainium2, 8 NeuronCores.

Problem: nn_Caps_47742856462336
  u:    [32, 1152, 16] f32
  W:    [1, 32, 1152, 32, 16] f32
  bias: [1, 32, 32] f32
  out = 2-iter dynamic routing -> [32, 32, 32] f32

Sharding: tensor-parallel over in_caps (k): 1152/8 = 144 per core. Routing
state is combined with small bf16 AllReduces (j-halves). All cores end with
identical output.

v3 design: materialize u_hat during the pre-collective barrier window.
  The first collective pays a ~55us all-core rendezvous (start skew +
  CC init) that nothing on this core can shrink; everything issued before
  it is effectively free. So:

  Pre-barrier (0..~55us):
    u_hat[b, j, k, o] for the local k-slice via 72 wide PE matmuls:
      stationary ud[t] chunk [(k8,i), (k8',bh)] = u * delta(k8==k8')
      moving    ws0 chunk    [(k8,i), (j,o)]
      -> psum   [(k8',bh), (j,o)]  (t = b//16, bh = b%16, k = 8*cc+k8')
    s0 = sum_k u_hat: DVE reduce over cc + tiny PE partition-pack
    -> AllReduce (j-halves, bf16)

  Post-AR0 (everything in the [b | (j,o)] / [(k8,bh) | (cc,j,o)] layouts,
  no transposes, softmax j-axis fully in the free dim):
    v0 = squash(s0/32 + bias)            [32, (j,o)]
    v0bc: PE partition-broadcast to [(k8,bh), (j,o)]
    A   = sum_o u_hat * v0bc             DVE mul + group-reduce
    c   = softmax_j(A)                   scalar Exp + DVE sums
    cu  = u_hat * c   (in-place on u_hat)
    s1p = sum_cc cu, PE partition-pack -> AllReduce (j-halves)
    out = squash(s1 + bias)              [32, (j,o)] f32
"""

import os
import sys
import numpy as np

for _p in ("/opt/trn_rl_repo", os.path.expanduser("~/.axon_site/_ro/trn_rl_repo")):
    if os.path.isdir(_p) and _p not in sys.path:
        sys.path.insert(0, _p)

import ml_dtypes  # noqa: E402

BF = ml_dtypes.bfloat16

B = 32      # batch
J = 32      # out_caps
O = 32      # out_dim
I = 16      # in_dim
KG = 1152   # global in_caps
NC = 8      # cores
KL = KG // NC   # 144 in_caps per core
KI = KL * I     # 2304 contraction size per core
NCH = KI // 128  # 18 chunks of 128 (k8, i) rows
EPS = 1e-7

JO = J * O           # 1024
NT = 2               # batch halves (t = b // 16)
BH = B // NT         # 16
NG = 6               # A-mul scratch groups (NCH/3 chunks each)
GC = NCH // NG       # 3 chunks per group


# ---------------------------------------------------------------------------
# host-side data prep: per-core DMA-friendly bf16/f32 layouts
# ---------------------------------------------------------------------------

def host_prep(u, W, bias):
    """Returns list of 8 dicts of named np arrays (the per-core DRAM inputs)."""
    u = np.asarray(u, dtype=np.float32)
    W = np.asarray(W, dtype=np.float32)
    bias = np.asarray(bias, dtype=np.float32)
    Wf = W[0]                      # [J, KG, O, I]
    biasf = bias[0]                # [J, O]

    # biasr [32, (j, o)] f32: bias broadcast over batch rows
    biasr = np.ascontiguousarray(
        np.broadcast_to(biasf.reshape(1, JO), (B, JO)), dtype=np.float32)

    # selbc [32, (t, k8, bh)]: v0 partition-broadcast (b' == 16t + bh)
    selbc = np.zeros((B, NT, 8, BH), dtype=np.float32)
    for t in range(NT):
        for bh in range(BH):
            selbc[BH * t + bh, t, :, bh] = 1.0
    selbc = selbc.reshape(B, NT * 128).astype(BF)

    # selpk [(k8, bh), (t, bh')]: sum over k8, scatter to b = 16t + bh rows
    selpk = np.zeros((8, BH, NT, BH), dtype=np.float32)
    for t in range(NT):
        for bh in range(BH):
            selpk[:, bh, t, bh] = 1.0
    selpk = selpk.reshape(128, NT * BH).astype(BF)

    ins = []
    for c in range(NC):
        ks = c * KL
        Wc = Wf[:, ks:ks + KL]                 # [J, KL, O, I]
        uc = u[:, ks:ks + KL]                  # [B, KL, I]

        # ws0 [128, (chunk, j, o)]: chunk rows (k8, i), free (j, o)
        ws0 = Wc.transpose(1, 3, 0, 2).reshape(KI, JO)       # [(k i), (j o)]
        ws0 = ws0.reshape(NCH, 128, JO).transpose(1, 0, 2).reshape(128, NCH * JO)

        # ud [128=(k8,i), (t, cc, k8', bh)] = u[16t+bh, 8cc+k8, i] d(k8==k8')
        um = uc.reshape(NT, BH, NCH, 8, I)     # [t, bh, cc, k8, i]
        ud = np.zeros((8, I, NT, NCH, 8, BH), dtype=np.float32)
        for k8 in range(8):
            # [i, t?, cc, bh] <- um[t, bh, cc, k8, i]
            ud[k8, :, :, :, k8, :] = um[:, :, :, k8, :].transpose(3, 0, 2, 1)
        ud = ud.reshape(128, NT * NCH * 128).astype(BF)

        ins.append({
            "ws0": np.ascontiguousarray(ws0).astype(BF),
            "ud": np.ascontiguousarray(ud),
            "selbc": selbc,
            "selpk": selpk,
            "biasr": biasr,
        })
    return ins


def host_unpack(out):
    """out [b, (j, o)] f32 -> [B, J, O]."""
    return np.ascontiguousarray(out.reshape(B, J, O))


# ---------------------------------------------------------------------------
# device program
# ---------------------------------------------------------------------------

def build_program(tc, outs, ins, n_cores=NC, use_cc=True, stop_after=None):
    import concourse.bass as bass  # noqa: F401
    from concourse import mybir

    F32 = mybir.dt.float32
    BF16 = mybir.dt.bfloat16
    ADD = mybir.AluOpType.add
    MULT = mybir.AluOpType.mult
    AX = mybir.AxisListType.X
    ACT = mybir.ActivationFunctionType

    nc = tc.nc
    ws0_d = ins["ws0"]; ud_d = ins["ud"]
    selbc_d = ins["selbc"]; selpk_d = ins["selpk"]; biasr_d = ins["biasr"]
    out_d = outs["out"]

    import contextlib
    stack = contextlib.ExitStack()
    with stack:
        pool = stack.enter_context(tc.tile_pool(name="main", bufs=1))
        big = stack.enter_context(tc.tile_pool(name="big", bufs=1))
        psum = stack.enter_context(tc.tile_pool(name="psum", bufs=1, space="PSUM"))
        dram = stack.enter_context(tc.tile_pool(name="dram", bufs=1, space="DRAM"))

        # ---- resident inputs (DMA issue order = priority order) -------------
        ws0 = big.tile([128, NCH * JO], BF16)
        ud = pool.tile([128, NT * NCH * 128], BF16)
        selbc = pool.tile([B, NT * 128], BF16)
        selpk = pool.tile([128, NT * BH], BF16)
        biasr = pool.tile([B, JO], F32)

        ws0v = ws0[:].rearrange("p (c f) -> p c f", c=NCH)
        ws0dv = ws0_d.rearrange("p (c f) -> p c f", c=NCH)
        for (a, b) in ((0, 3), (3, 6), (6, 9), (9, 12), (12, 15), (15, NCH)):
            nc.sync.dma_start(ws0v[:, a:b], ws0dv[:, a:b])
        nc.sync.dma_start(ud[:], ud_d)
        for tile_, dram_ in ((selbc, selbc_d), (selpk, selpk_d),
                             (biasr, biasr_d)):
            nc.sync.dma_start(tile_[:], dram_)
        udv = ud[:].rearrange("p (t c f) -> p t c f", t=NT, c=NCH)
        selbcv = selbc[:].rearrange("b (t f) -> b t f", t=NT)
        selpkv = selpk[:].rearrange("p (t f) -> p t f", t=NT)

        # ---- collective bounce buffers (bf16, j-halves) ---------------------
        rg = [list(range(n_cores))]
        cc0i = [dram.tile([B, 512], BF16, name=f"cc0i{h}") for h in range(2)]
        cc0o = [dram.tile([B, 512], BF16, name=f"cc0o{h}", addr_space="Shared")
                for h in range(2)]
        cc1i = [dram.tile([B, 512], BF16, name=f"cc1i{h}") for h in range(2)]
        cc1o = [dram.tile([B, 512], BF16, name=f"cc1o{h}", addr_space="Shared")
                for h in range(2)]

        def _finish(tile_ap, rows):
            """Timing-bisect helper: route a dependency on `tile_ap` to out."""
            z = pool.tile([B, JO], F32, tag="finz")
            nc.vector.memset(z[:], 0.0)
            nc.vector.tensor_copy(z[:rows, :1], tile_ap[:rows, :1])
            nc.scalar.dma_start(out_d, z[:])

        # ---- u_hat: 72 wide matmuls, psum -> bf16 sbuf ----------------------
        # uh[t] [(k8,bh), (cc, j, o)]
        uh = [big.tile([128, NCH * JO], BF16, name=f"uh{t}") for t in range(NT)]
        uhv = [uh[t][:].rearrange("p (c f) -> p c f", c=NCH) for t in range(NT)]
        for t in range(NT):
            for cc in range(NCH):
                for h2 in range(2):
                    pu = psum.tile([128, 512], F32, tag="uhps", bufs=4)
                    nc.tensor.matmul(
                        pu[:], udv[:, t, cc], ws0v[:, cc, 512 * h2:512 * h2 + 512],
                        start=True, stop=True)
                    # split psum->sbuf copies across scalar+vector engines
                    if (cc + h2) % 2 == 0:
                        nc.scalar.activation(
                            uhv[t][:, cc, 512 * h2:512 * h2 + 512], pu[:],
                            ACT.Copy)
                    else:
                        nc.vector.tensor_copy(
                            uhv[t][:, cc, 512 * h2:512 * h2 + 512], pu[:])

        if stop_after == "uh":
            return _finish(uh[1][:], B)

        # ---- s0 = sum_k u_hat -> [32, (j, o)] -> AllReduce (j-halves) -------
        s0pb = [pool.tile([128, JO], BF16, name=f"s0pb{t}") for t in range(NT)]
        for h in range(2):
            hs = slice(512 * h, 512 * h + 512)
            pp = psum.tile([B, 512], F32, tag="pkps", bufs=2, name=f"s0pk{h}")
            for t in range(NT):
                s0p = pool.tile([128, 512], F32, tag="s0p", bufs=2)
                nc.vector.tensor_reduce(
                    s0p[:],
                    uhv[t][:, :, hs].rearrange("p c f -> p f c"),
                    axis=AX, op=ADD)
                nc.vector.tensor_copy(s0pb[t][:, hs], s0p[:])
                # rows 16t..16t+16 of pp get this t's k8-sum (t IS the b-split)
                nc.tensor.matmul(pp[BH * t:BH * t + BH, :], selpkv[:, t],
                                 s0pb[t][:, hs], start=True, stop=True)
            s0r = pool.tile([B, 512], BF16, tag="s0r", bufs=2)
            nc.scalar.activation(s0r[:], pp[:], ACT.Copy)
            if use_cc:
                nc.scalar.dma_start(cc0i[h][:], s0r[:])
                nc.gpsimd.collective_compute(
                    "AllReduce", ADD, replica_groups=rg,
                    ins=[cc0i[h].opt()], outs=[cc0o[h].opt()])
            else:
                nc.scalar.dma_start(cc0o[h][:], s0r[:])

        if stop_after == "s0":
            return _finish(s0pb[1][:], B)

        # ---- squash helper (rows=32, j in free dim) -------------------------
        epsb = pool.tile([B, 1], F32)
        nc.vector.memset(epsb[:], EPS)

        def squash_m(src, nj, tag):
            """m[32, nj]: per-(b, j) squash scale factor of src [32, (j, o)]."""
            t_ = pool.tile([B, nj * O], F32, tag=f"sq_t{tag}", bufs=2)
            nc.vector.tensor_mul(t_[:], src, src)
            sq = pool.tile([B, nj], F32, tag=f"sq_s{tag}", bufs=2)
            nc.vector.tensor_reduce(
                sq[:], t_[:].rearrange("p (j o) -> p j o", o=O), axis=AX, op=ADD)
            one = pool.tile([B, nj], F32, tag=f"sq_o{tag}", bufs=2)
            nc.vector.tensor_scalar_add(one[:], sq[:], 1.0)
            sqr = pool.tile([B, nj], F32, tag=f"sq_r{tag}", bufs=2)
            nc.scalar.activation(sqr[:], sq[:], ACT.Sqrt, bias=epsb[:B])
            den = pool.tile([B, nj], F32, tag=f"sq_d{tag}", bufs=2)
            nc.vector.tensor_mul(den[:], one[:], sqr[:])
            r = pool.tile([B, nj], F32, tag=f"sq_rr{tag}", bufs=2)
            nc.vector.reciprocal(r[:], den[:])
            m = pool.tile([B, nj], F32, tag=f"sq_m{tag}", bufs=2)
            nc.vector.tensor_mul(m[:], sq[:], r[:])
            return m

        # ---- v0 halves: squash(s0/32 + bias) -> PE bcast to (k8,bh) rows ----
        v0bc = [pool.tile([128, JO], BF16, name=f"v0bc{t}") for t in range(NT)]
        for h in range(2):
            hs = slice(512 * h, 512 * h + 512)
            if use_cc:
                s0g = pool.tile([B, 512], BF16, tag="s0g", bufs=2)
                nc.scalar.dma_start(s0g[:], cc0o[h][:])
            else:
                s0g = pool.tile([B, 512], BF16, tag="s0g", bufs=2)
                nc.scalar.dma_start(s0g[:], cc0o[h][:])
            s0f = pool.tile([B, 512], F32, tag="s0f", bufs=2)
            nc.vector.scalar_tensor_tensor(
                s0f[:], s0g[:], 1.0 / 32.0, biasr[:, hs], op0=MULT, op1=ADD)
            m = squash_m(s0f[:], 16, tag="v0")
            v0r = pool.tile([B, 512], BF16, tag="v0r", bufs=2)
            mv = m[:].unsqueeze(2).broadcast_to((B, 16, O))
            nc.vector.tensor_mul(
                v0r[:].rearrange("p (j o) -> p j o", o=O),
                s0f[:].rearrange("p (j o) -> p j o", o=O), mv)
            for t in range(NT):
                pv = psum.tile([128, 512], F32, tag="vbps", bufs=2)
                nc.tensor.matmul(pv[:], selbcv[:, t], v0r[:],
                                 start=True, stop=True)
                nc.scalar.activation(v0bc[t][:, hs], pv[:], ACT.Copy)

        if stop_after == "v0":
            return _finish(v0bc[1][:], 128)

        # ---- A = sum_o u_hat * v0bc; groups keep the scratch small ----------
        A = [pool.tile([128, NCH * J], F32, name=f"A{t}") for t in range(NT)]
        for t in range(NT):
            for g in range(NG):
                cs = slice(GC * g, GC * g + GC)
                am = pool.tile([128, GC * JO], BF16, tag="am", bufs=2)
                amv = am[:].rearrange("p (c j o) -> p c j o", c=GC, j=J)
                nc.vector.tensor_mul(
                    amv, uhv[t][:, cs].rearrange("p c (j o) -> p c j o", j=J),
                    v0bc[t][:].rearrange("p (j o) -> p j o", o=O)
                    .unsqueeze(1).broadcast_to((128, GC, J, O)))
                nc.vector.tensor_reduce(
                    A[t][:, J * GC * g:J * GC * (g + 1)],
                    amv.rearrange("p c j o -> p (c j) o"),
                    axis=AX, op=ADD)

        if stop_after == "A":
            return _finish(A[1][:], 128)

        # ---- softmax over j (free dim): exp, sum_j, recip, scale ------------
        c1 = [pool.tile([128, NCH * J], BF16, name=f"c1{t}") for t in range(NT)]
        for t in range(NT):
            E = pool.tile([128, NCH * J], F32, tag="sme", bufs=2)
            nc.scalar.activation(E[:], A[t][:], ACT.Exp)
            den = pool.tile([128, NCH], F32, tag="smd", bufs=2)
            nc.vector.tensor_reduce(
                den[:], E[:].rearrange("p (c j) -> p c j", c=NCH),
                axis=AX, op=ADD)
            r = pool.tile([128, NCH], F32, tag="smr", bufs=2)
            nc.vector.reciprocal(r[:], den[:])
            nc.vector.tensor_mul(
                c1[t][:].rearrange("p (c j) -> p c j", c=NCH),
                E[:].rearrange("p (c j) -> p c j", c=NCH),
                r[:].unsqueeze(2).broadcast_to((128, NCH, J)))

        if stop_after == "c1":
            return _finish(c1[1][:], 128)

        # ---- cu = u_hat * c (in-place); s1 = sum_k cu -> AllReduce ----------
        s1pb = [pool.tile([128, JO], BF16, name=f"s1pb{t}") for t in range(NT)]
        for h in range(2):
            js = slice(16 * h, 16 * h + 16)
            hs = slice(512 * h, 512 * h + 512)
            for t in range(NT):
                uhj = uhv[t].rearrange("p c (j o) -> p c j o", j=J)[:, :, js]
                nc.vector.tensor_mul(
                    uhj, uhj,
                    c1[t][:].rearrange("p (c j) -> p c j", c=NCH)[:, :, js]
                    .unsqueeze(3).broadcast_to((128, NCH, 16, O)))
            pp = psum.tile([B, 512], F32, tag="pkps", bufs=2, name=f"s1pk{h}")
            for t in range(NT):
                s1p = pool.tile([128, 512], F32, tag="s1p", bufs=2)
                nc.vector.tensor_reduce(
                    s1p[:],
                    uhv[t][:, :, hs].rearrange("p c f -> p f c"),
                    axis=AX, op=ADD)
                nc.vector.tensor_copy(s1pb[t][:, hs], s1p[:])
                nc.tensor.matmul(pp[BH * t:BH * t + BH, :], selpkv[:, t],
                                 s1pb[t][:, hs], start=True, stop=True)
            s1r = pool.tile([B, 512], BF16, tag="s1r", bufs=2)
            nc.scalar.activation(s1r[:], pp[:], ACT.Copy)
            if use_cc:
                nc.scalar.dma_start(cc1i[h][:], s1r[:])
                nc.gpsimd.collective_compute(
                    "AllReduce", ADD, replica_groups=rg,
                    ins=[cc1i[h].opt()], outs=[cc1o[h].opt()])
            else:
                nc.scalar.dma_start(cc1o[h][:], s1r[:])

        if stop_after == "s1":
            return _finish(s1pb[1][:], B)

        # ---- out halves: squash(s1 + bias) ----------------------------------
        for h in range(2):
            hs = slice(512 * h, 512 * h + 512)
            s1g = pool.tile([B, 512], BF16, tag="s1g", bufs=2)
            nc.scalar.dma_start(s1g[:], cc1o[h][:])
            s1f = pool.tile([B, 512], F32, tag="s1f", bufs=2)
            nc.vector.tensor_add(s1f[:], s1g[:], biasr[:, hs])
            m = squash_m(s1f[:], 16, tag="v1")
            v1 = pool.tile([B, 512], F32, tag="v1", bufs=2)
            mv = m[:].unsqueeze(2).broadcast_to((B, 16, O))
            nc.vector.tensor_mul(
                v1[:].rearrange("p (j o) -> p j o", o=O),
                s1f[:].rearrange("p (j o) -> p j o", o=O), mv)
            nc.scalar.dma_start(out_d[:, hs], v1[:])


# ---------------------------------------------------------------------------
# compile + run
# ---------------------------------------------------------------------------

_CACHE = {}


def _get_compiled(use_cc=True, n_cores=NC):
    key = (use_cc, n_cores)
    if key in _CACHE:
        return _CACHE[key]
    import concourse.bacc as bacc
    import concourse.tile as tile
    from concourse import mybir

    nc = bacc.Bacc("TRN2", target_bir_lowering=False, debug=False,
                   num_devices=n_cores)
    F32 = mybir.dt.float32
    BF16 = mybir.dt.bfloat16
    shapes = {
        "ws0": ([128, NCH * JO], BF16),
        "ud": ([128, NT * NCH * 128], BF16),
        "selbc": ([B, NT * 128], BF16),
        "selpk": ([128, NT * BH], BF16),
        "biasr": ([B, JO], F32),
    }
    ins = {k: nc.dram_tensor(k, sh, dt, kind="ExternalInput").ap()
           for k, (sh, dt) in shapes.items()}
    outs = {"out": nc.dram_tensor("out", [B, JO], F32,
                                  kind="ExternalOutput").ap()}
    with tile.TileContext(nc) as tc:
        build_program(tc, outs, ins, n_cores=n_cores, use_cc=use_cc)
    nc.compile()
    _CACHE[key] = nc
    return nc


def kernel(**inputs):
    from concourse import bass_utils

    in_maps = host_prep(inputs["u"], inputs["W"], inputs["bias"])
    nc = _get_compiled()
    res = bass_utils.run_bass_kernel_spmd(nc, in_maps, core_ids=list(range(NC)))
    return host_unpack(np.asarray(res.results[0]["out"], dtype=np.float32))


# revision 13
# speedup vs baseline: 1.2385x; 1.2385x over previous
"""CapsNet dynamic-routing kernel for Trainium2, 8 NeuronCores.

Problem: nn_Caps_47742856462336
  u:    [32, 1152, 16] f32
  W:    [1, 32, 1152, 32, 16] f32
  bias: [1, 32, 32] f32
  out = 2-iter dynamic routing -> [32, 32, 32] f32

v5 design: tensor-parallel over OUT_CAPS (j): 4 per core, full k=1152
locally. The routing softmax over j is the ONLY cross-core dependency,
and it only needs the per-(b,k) sum of exp(A) -- so the kernel has exactly
ONE collective (a 74KB bf16 AllReduce of softmax denominators) instead of
the two s0/s1 rounds a k-sharded layout needs. s0, v0, A, c, s1, out are
all local; the host concatenates the 8 per-core j-slices.

Per core:
  u_hat[b, j4, k, o] via 288 PE matmuls (contraction (k8,i)-chunks on
    partitions; stationary = block-diag u so k8 stays resolved):
      ud chunk [(k8,i), (k8',bh)], moving wsj chunk [(k8,i), (j4,o)]
      -> psum [(k8',bh), (j4,o)], t = b//16, bh = b%16, k = 8cc+k8'
    ud is built on-device (memset + 8 partition-group copies) from a
    compact u input to keep DMA small; t-halves reuse one ud buffer.
  s0 = sum_k u_hat: PE accumulating pack-matmuls (stationary selpk).
  v0 = squash(s0/32 + bias)  [16=bh, (t,j4,o)], PE-broadcast to k8 rows.
  A = sum_o u_hat*v0  (DVE mul long-run + DVE/GpSimd group reduce)
  den_part = sum_{j4} exp(A)  ->  AllReduce  (the one collective)
  c = exp(A) / den;  cu = u_hat*c in-place;  s1 = sum_k cu (PE pack)
  out = squash(s1 + bias)  [16, (t,j4,o)] f32.
"""

import os
import sys
import numpy as np

for _p in ("/opt/trn_rl_repo", os.path.expanduser("~/.axon_site/_ro/trn_rl_repo")):
    if os.path.isdir(_p) and _p not in sys.path:
        sys.path.insert(0, _p)

import ml_dtypes  # noqa: E402

BF = ml_dtypes.bfloat16

B = 32      # batch
J = 32      # out_caps
O = 32      # out_dim
I = 16      # in_dim
KG = 1152   # in_caps (full, per core)
NC = 8      # cores
JL = J // NC    # 4 out_caps per core
NCH = KG // 8   # 144 chunks of 128 = (k8, i) rows
EPS = 1e-7

JO4 = JL * O         # 128
NT = 2               # batch halves (t = b // 16)
BH = B // NT         # 16
GC = 12              # chunks per A-mul scratch group
NG = NCH // GC       # 12 groups


# ---------------------------------------------------------------------------
# host-side data prep
# ---------------------------------------------------------------------------

def host_prep(u, W, bias):
    """Returns list of 8 dicts of named np arrays (the per-core DRAM inputs)."""
    u = np.asarray(u, dtype=np.float32)
    W = np.asarray(W, dtype=np.float32)
    bias = np.asarray(bias, dtype=np.float32)
    Wf = W[0]                      # [J, KG, O, I]
    biasf = bias[0]                # [J, O]

    # urows [128=(k8,i), (t, cc, bh)] = u[16t+bh, 8cc+k8, i]  (compact)
    um = u.reshape(NT, BH, NCH, 8, I)              # [t, bh, cc, k8, i]
    urows = um.transpose(3, 4, 0, 2, 1).reshape(128, NT * NCH * BH)

    # selpk [(k8, bh), bh'] = d(bh == bh'): sum over k8 partitions
    selpk = np.zeros((8, BH, BH), dtype=np.float32)
    for bh in range(BH):
        selpk[:, bh, bh] = 1.0
    selpk = selpk.reshape(128, BH).astype(BF)

    # selbc [16=bh', (t, k8, bh)] = d(bh' == bh): v0 partition-broadcast
    selbc = np.zeros((BH, NT, 8, BH), dtype=np.float32)
    for bh in range(BH):
        selbc[bh, :, :, bh] = 1.0
    selbc = selbc.reshape(BH, NT * 128).astype(BF)

    ins = []
    for c in range(NC):
        js = c * JL
        Wc = Wf[js:js + JL]                        # [JL, KG, O, I]

        # wsj [128, (cc, j4, o)]: chunk rows (k8, i), free (j4, o)
        wsj = Wc.transpose(1, 3, 0, 2).reshape(KG * I, JO4)   # [(k i), (j4 o)]
        wsj = wsj.reshape(NCH, 128, JO4).transpose(1, 0, 2).reshape(128, NCH * JO4)

        # biasj [16=bh, (t, j4, o)] f32: bias j-slice bcast over (bh, t)
        bj = np.broadcast_to(biasf[js:js + JL].reshape(1, 1, JL, O),
                             (BH, NT, JL, O))
        biasj = np.ascontiguousarray(bj.reshape(BH, NT * JO4), dtype=np.float32)

        ins.append({
            "wsj": np.ascontiguousarray(wsj).astype(BF),
            "urows": np.ascontiguousarray(urows).astype(BF),
            "selpk": selpk,
            "selbc": selbc,
            "biasj": biasj,
        })
    return ins


def host_unpack(outs_list):
    """8 per-core outs [16=bh, (t, j4, o)] f32 -> [B, J, O]."""
    slabs = []
    for arr in outs_list:
        a = np.asarray(arr, np.float32).reshape(BH, NT, JL, O)
        slabs.append(a.transpose(1, 0, 2, 3).reshape(B, JL, O))
    return np.ascontiguousarray(np.concatenate(slabs, axis=1))


# ---------------------------------------------------------------------------
# device program
# ---------------------------------------------------------------------------

def build_program(tc, outs, ins, n_cores=NC, use_cc=True, stop_after=None):
    import concourse.bass as bass  # noqa: F401
    from concourse import mybir

    F32 = mybir.dt.float32
    BF16 = mybir.dt.bfloat16
    ADD = mybir.AluOpType.add
    MULT = mybir.AluOpType.mult
    AX = mybir.AxisListType.X
    ACT = mybir.ActivationFunctionType

    nc = tc.nc
    wsj_d = ins["wsj"]; urows_d = ins["urows"]
    selpk_d = ins["selpk"]; selbc_d = ins["selbc"]; biasj_d = ins["biasj"]
    out_d = outs["out"]

    import contextlib
    stack = contextlib.ExitStack()
    with stack:
        pool = stack.enter_context(tc.tile_pool(name="main", bufs=1))
        big = stack.enter_context(tc.tile_pool(name="big", bufs=1))
        psum = stack.enter_context(tc.tile_pool(name="psum", bufs=1, space="PSUM"))
        dram = stack.enter_context(tc.tile_pool(name="dram", bufs=1, space="DRAM"))

        # ---- resident inputs -------------------------------------------------
        wsj = big.tile([128, NCH * JO4], BF16)
        urows = pool.tile([128, NT * NCH * BH], BF16)
        selpk = pool.tile([128, BH], BF16)
        selbc = pool.tile([BH, NT * 128], BF16)
        biasj = pool.tile([BH, NT * JO4], F32)

        wsjv = wsj[:].rearrange("p (c f) -> p c f", c=NCH)
        wsjdv = wsj_d.rearrange("p (c f) -> p c f", c=NCH)
        for (a, b) in ((0, 24), (24, 48), (48, 72), (72, 96), (96, 120),
                       (120, NCH)):
            nc.sync.dma_start(wsjv[:, a:b], wsjdv[:, a:b])
        nc.sync.dma_start(urows[:], urows_d)
        for tile_, dram_ in ((selpk, selpk_d), (selbc, selbc_d),
                             (biasj, biasj_d)):
            nc.sync.dma_start(tile_[:], dram_)
        urv = urows[:].rearrange("p (t c bh) -> p t c bh", t=NT, c=NCH)
        selbcv = selbc[:].rearrange("b (t f) -> b t f", t=NT)

        # ---- collective bounce (softmax denominators, bf16) -----------------
        rg = [list(range(n_cores))]
        ccdi = dram.tile([128, NT * NCH], BF16, name="ccdi")
        ccdo = dram.tile([128, NT * NCH], BF16, name="ccdo",
                         addr_space="Shared")

        def _finish(tile_ap, rows):
            z = pool.tile([BH, NT * JO4], F32, tag="finz")
            nc.vector.memset(z[:], 0.0)
            nc.vector.tensor_copy(z[:rows, :1], tile_ap[:rows, :1])
            nc.scalar.dma_start(out_d, z[:])

        # ---- ud diag build + u_hat matmuls + s0 pack, per t-half ------------
        # ud [128=(k8,i), (cc, k8', bh)]; uh[t] [128=(k8,bh), (cc, j4, o)]
        ud = big.tile([128, NCH * 128], BF16)
        nc.vector.memset(ud[:], 0.0)
        udv = ud[:].rearrange("p (c f) -> p c f", c=NCH)
        uh = [big.tile([128, NCH * JO4], BF16, name=f"uh{t}") for t in range(NT)]
        uhv = [uh[t][:].rearrange("p (c f) -> p c f", c=NCH) for t in range(NT)]
        s0 = pool.tile([BH, NT * JO4], BF16, name="s0")
        for t in range(NT):
            # diag fill via sbuf->sbuf DMA (no partition-offset limits)
            for k8 in range(8):
                dst = udv[16 * k8:16 * k8 + 16, :, 16 * k8:16 * k8 + 16]
                src = urv[16 * k8:16 * k8 + 16, t]
                nc.sync.dma_start(dst, src)
            ps0 = psum.tile([BH, JO4], F32, tag="s0ps", bufs=1)
            for c4 in range(NCH // 4):
                pu = psum.tile([128, 512], F32, tag="uhps", bufs=3)
                for g in range(4):
                    cc = 4 * c4 + g
                    nc.tensor.matmul(
                        pu[:, JO4 * g:JO4 * g + JO4], udv[:, cc], wsjv[:, cc],
                        start=True, stop=True)
                if c4 % 2 == 0:
                    nc.scalar.activation(
                        uhv[t][:, 4 * c4:4 * c4 + 4].rearrange("p c f -> p (c f)"),
                        pu[:], ACT.Copy)
                else:
                    nc.vector.tensor_copy(
                        uhv[t][:, 4 * c4:4 * c4 + 4].rearrange("p c f -> p (c f)"),
                        pu[:])
                for g in range(4):
                    cc = 4 * c4 + g
                    nc.tensor.matmul(ps0[:], selpk[:], uhv[t][:, cc],
                                     start=(cc == 0), stop=(cc == NCH - 1))
            nc.scalar.activation(s0[:, JO4 * t:JO4 * t + JO4], ps0[:], ACT.Copy)

        if stop_after == "uh":
            return _finish(uh[1][:], BH)

        # ---- squash helper (rows=16, (t,j4) groups in free dim) -------------
        epsb = pool.tile([BH, 1], F32)
        nc.vector.memset(epsb[:], EPS)

        def squash_m(src, nj, tag):
            t_ = pool.tile([BH, nj * O], F32, tag=f"sq_t{tag}", bufs=2)
            nc.vector.tensor_mul(t_[:], src, src)
            sq = pool.tile([BH, nj], F32, tag=f"sq_s{tag}", bufs=2)
            nc.vector.tensor_reduce(
                sq[:], t_[:].rearrange("p (j o) -> p j o", o=O), axis=AX, op=ADD)
            one = pool.tile([BH, nj], F32, tag=f"sq_o{tag}", bufs=2)
            nc.vector.tensor_scalar_add(one[:], sq[:], 1.0)
            sqr = pool.tile([BH, nj], F32, tag=f"sq_r{tag}", bufs=2)
            nc.scalar.activation(sqr[:], sq[:], ACT.Sqrt, bias=epsb[:BH])
            den = pool.tile([BH, nj], F32, tag=f"sq_d{tag}", bufs=2)
            nc.vector.tensor_mul(den[:], one[:], sqr[:])
            r = pool.tile([BH, nj], F32, tag=f"sq_rr{tag}", bufs=2)
            nc.vector.reciprocal(r[:], den[:])
            m = pool.tile([BH, nj], F32, tag=f"sq_m{tag}", bufs=2)
            nc.vector.tensor_mul(m[:], sq[:], r[:])
            return m

        # ---- v0 = squash(s0/32 + bias) -> PE bcast to (k8,bh) rows ----------
        s0f = pool.tile([BH, NT * JO4], F32, name="s0f")
        nc.vector.scalar_tensor_tensor(
            s0f[:], s0[:], 1.0 / 32.0, biasj[:], op0=MULT, op1=ADD)
        m = squash_m(s0f[:], NT * JL, tag="v0")
        v0r = pool.tile([BH, NT * JO4], BF16, name="v0r")
        mv = m[:].unsqueeze(2).broadcast_to((BH, NT * JL, O))
        nc.vector.tensor_mul(
            v0r[:].rearrange("p (j o) -> p j o", o=O),
            s0f[:].rearrange("p (j o) -> p j o", o=O), mv)
        v0bc = [pool.tile([128, JO4], BF16, name=f"v0bc{t}") for t in range(NT)]
        for t in range(NT):
            pv = psum.tile([128, JO4], F32, tag="vbps", bufs=1)
            nc.tensor.matmul(pv[:], selbcv[:, t],
                             v0r[:, JO4 * t:JO4 * t + JO4],
                             start=True, stop=True)
            nc.scalar.activation(v0bc[t][:], pv[:], ACT.Copy)

        if stop_after == "v0":
            return _finish(v0bc[1][:], BH)

        # ---- A = sum_o u_hat * v0bc; exp; partial den -> AllReduce ----------
        # A[t] [128, (cc, j4)]; E[t] = exp(A); denp [128, (t, cc)]
        E = [pool.tile([128, NCH * JL], BF16, name=f"E{t}") for t in range(NT)]
        A = [pool.tile([128, NCH * JL], F32, name=f"A{t}") for t in range(NT)]
        denp = pool.tile([128, NT * NCH], F32, name="denp")
        for t in range(NT):
            for g in range(NG):
                cs = slice(GC * g, GC * g + GC)
                am = pool.tile([128, GC * JO4], BF16, tag="am", bufs=1)
                # long-run mul: src0 contiguous, src1 bcast over cc (outer)
                veng = nc.vector if g % 2 == 0 else nc.gpsimd
                veng.tensor_mul(
                    am[:].rearrange("p (c f) -> p c f", c=GC),
                    uhv[t][:, cs],
                    v0bc[t][:].unsqueeze(1).broadcast_to((128, GC, JO4)))
                veng.tensor_reduce(
                    A[t][:, JL * GC * g:JL * GC * (g + 1)],
                    am[:].rearrange("p (cj o) -> p cj o", o=O),
                    axis=AX, op=ADD)
            nc.scalar.activation(E[t][:], A[t][:], ACT.Exp)
            nc.vector.tensor_reduce(
                denp[:, NCH * t:NCH * t + NCH],
                E[t][:].rearrange("p (c j) -> p c j", c=NCH),
                axis=AX, op=ADD)
        denb = pool.tile([128, NT * NCH], BF16, name="denb")
        nc.vector.tensor_copy(denb[:], denp[:])
        if use_cc:
            nc.scalar.dma_start(ccdi[:], denb[:])
            nc.gpsimd.collective_compute(
                "AllReduce", ADD, replica_groups=rg,
                ins=[ccdi.opt()], outs=[ccdo.opt()])
        else:
            nc.scalar.dma_start(ccdo[:], denb[:])

        if stop_after == "A":
            return _finish(A[1][:], 128)

        # ---- softmax finalize: c = E / den; cu in-place; s1 pack ------------
        deng = pool.tile([128, NT * NCH], BF16, tag="deng")
        nc.scalar.dma_start(deng[:], ccdo[:])
        dengv = deng[:].rearrange("p (t c) -> p t c", t=NT)
        s1 = pool.tile([BH, NT * JO4], BF16, name="s1")
        for t in range(NT):
            r = pool.tile([128, NCH], F32, tag="smr", bufs=2)
            nc.vector.reciprocal(r[:], dengv[:, t])
            # c[t] = E[t] * r  (overwrite E)
            nc.vector.tensor_mul(
                E[t][:].rearrange("p (c j) -> p c j", c=NCH),
                E[t][:].rearrange("p (c j) -> p c j", c=NCH),
                r[:].unsqueeze(2).broadcast_to((128, NCH, JL)))
            # cu = u_hat * c (in-place on uh), split vector/gpsimd by half
            half = NCH // 2
            for (eng, cs) in ((nc.vector, slice(0, half)),
                              (nc.gpsimd, slice(half, NCH))):
                eng.tensor_mul(
                    uhv[t][:, cs].rearrange("p c (j o) -> p c j o", j=JL),
                    uhv[t][:, cs].rearrange("p c (j o) -> p c j o", j=JL),
                    E[t][:].rearrange("p (c j) -> p c j", c=NCH)[:, cs]
                    .unsqueeze(3).broadcast_to((128, half, JL, O)))
            ps1 = psum.tile([BH, JO4], F32, tag="s1ps", bufs=1)
            for cc in range(NCH):
                nc.tensor.matmul(ps1[:], selpk[:], uhv[t][:, cc],
                                 start=(cc == 0), stop=(cc == NCH - 1))
            nc.scalar.activation(s1[:, JO4 * t:JO4 * t + JO4], ps1[:], ACT.Copy)

        if stop_after == "s1":
            return _finish(s1[:], BH)

        # ---- out = squash(s1 + bias) ----------------------------------------
        s1f = pool.tile([BH, NT * JO4], F32, name="s1f")
        nc.vector.tensor_add(s1f[:], s1[:], biasj[:])
        m1 = squash_m(s1f[:], NT * JL, tag="v1")
        v1 = pool.tile([BH, NT * JO4], F32, name="v1")
        m1v = m1[:].unsqueeze(2).broadcast_to((BH, NT * JL, O))
        nc.vector.tensor_mul(
            v1[:].rearrange("p (j o) -> p j o", o=O),
            s1f[:].rearrange("p (j o) -> p j o", o=O), m1v)
        nc.scalar.dma_start(out_d, v1[:])


# ---------------------------------------------------------------------------
# compile + run
# ---------------------------------------------------------------------------

_CACHE = {}


def _get_compiled(use_cc=True, n_cores=NC):
    key = (use_cc, n_cores)
    if key in _CACHE:
        return _CACHE[key]
    import concourse.bacc as bacc
    import concourse.tile as tile
    from concourse import mybir

    nc = bacc.Bacc("TRN2", target_bir_lowering=False, debug=False,
                   num_devices=n_cores)
    F32 = mybir.dt.float32
    BF16 = mybir.dt.bfloat16
    shapes = {
        "wsj": ([128, NCH * JO4], BF16),
        "urows": ([128, NT * NCH * BH], BF16),
        "selpk": ([128, BH], BF16),
        "selbc": ([BH, NT * 128], BF16),
        "biasj": ([BH, NT * JO4], F32),
    }
    ins = {k: nc.dram_tensor(k, sh, dt, kind="ExternalInput").ap()
           for k, (sh, dt) in shapes.items()}
    outs = {"out": nc.dram_tensor("out", [BH, NT * JO4], F32,
                                  kind="ExternalOutput").ap()}
    with tile.TileContext(nc) as tc:
        build_program(tc, outs, ins, n_cores=n_cores, use_cc=use_cc)
    nc.compile()
    _CACHE[key] = nc
    return nc


def kernel(**inputs):
    from concourse import bass_utils

    in_maps = host_prep(inputs["u"], inputs["W"], inputs["bias"])
    nc = _get_compiled()
    res = bass_utils.run_bass_kernel_spmd(nc, in_maps, core_ids=list(range(NC)))
    return host_unpack([res.results[c]["out"] for c in range(NC)])


# revision 14
# speedup vs baseline: 2.0841x; 1.6828x over previous
"""CapsNet dynamic-routing kernel for Trainium2, 8 NeuronCores.

Problem: nn_Caps_47742856462336
  u:    [32, 1152, 16] f32
  W:    [1, 32, 1152, 32, 16] f32
  bias: [1, 32, 32] f32
  out = 2-iter dynamic routing -> [32, 32, 32] f32

Sharding: tensor-parallel over in_caps (k): 1152/8 = 144 per core. Routing
state is combined with four small bf16 AllReduces (two j8-half pairs), each
overlapped with neighbouring compute. All cores end with identical output.

v2 design (all row spaces use (j4, b) = 128 partitions, j4-outer):
  The contraction index (k, i) = 144*16 = 2304 is split into 18 chunks of
  128 partitions (rows (k8, i), i inner, k = 8*chunk + k8).

  s0   = sum_{k,i} u*W        chunked PE: stationary urep3, moving w_s0
  v0   = squash(s0/32 + bias) after AllReduce (split in j8-halves)
  Wv   chunks: stationary wo-chunk [(j4 o), 128(ki)], moving v0bd ->
         psum [(ki), (j4 b)]; ACT-copy to bf16, DVE-mul by urep3 (2x mode)
  A    = sum_i (Wv*u): PE blockdiag ones-reduce over i -> A[k, (j8 j4 b)]
         (k on partitions => softmax over j is free-dim only, no max pass)
  c1   = exp(A) / sum_j exp(A)   (|A| <~ 20, no overflow)
  cu   = c1*u built directly on k-partitions (no transposes)
  s1   = per-j8 i-plane matmuls (k 0..127) + 2 chunked matmuls (k 128..143,
         via PE partition-broadcast of c2) -> AllReduce -> squash -> out
"""

import os
import sys
import numpy as np

for _p in ("/opt/trn_rl_repo", os.path.expanduser("~/.axon_site/_ro/trn_rl_repo")):
    if os.path.isdir(_p) and _p not in sys.path:
        sys.path.insert(0, _p)

import ml_dtypes  # noqa: E402

BF = ml_dtypes.bfloat16

B = 32      # batch
J = 32      # out_caps
O = 32      # out_dim
I = 16      # in_dim
KG = 1152   # global in_caps
NC = 8      # cores
KL = KG // NC   # 144 in_caps per core
KI = KL * I     # 2304 contraction size per core
NCH = KI // 128  # 18 chunks of 128 (k8, i) rows
EPS = 1e-7

J8 = 8   # j // 4
J4 = 4   # j %  4
JO = J * O           # 1024
M128 = J4 * B        # 128 rows (j4, b), j4-outer
FJ = J8 * M128       # 1024 free (j8, j4, b)

SPLIT_AR0 = True     # AllReduce s0 in two j8-halves (overlap 2nd with v0/Wv)
SPLIT_AR1 = True     # AllReduce s1 in two j8-halves (overlap with squash)


# ---------------------------------------------------------------------------
# host-side data prep: per-core DMA-friendly bf16/f32 layouts
# ---------------------------------------------------------------------------

def host_prep(u, W, bias):
    """Returns list of 8 dicts of named np arrays (the per-core DRAM inputs)."""
    u = np.asarray(u, dtype=np.float32)
    W = np.asarray(W, dtype=np.float32)
    bias = np.asarray(bias, dtype=np.float32)
    Wf = W[0]                      # [J, KG, O, I]
    biasf = bias[0]                # [J, O]

    # bias1[(j4 b), (j8 o)] = biasf[4*j8+j4, o]
    b1 = biasf.reshape(J8, J4, O).transpose(1, 0, 2)          # [j4, j8, o]
    b1 = np.broadcast_to(b1.reshape(J4, 1, J8 * O), (J4, B, J8 * O))
    bias1 = np.ascontiguousarray(b1.reshape(J4 * B, J8 * O), dtype=np.float32)

    # ones64[p=(k8,i), r, m=(c_sub, k8')] = 1 iff c_sub==r and k8'==p//16
    ones64 = np.zeros((128, 8, 64), dtype=np.float32)
    for p in range(128):
        for r in range(8):
            ones64[p, r, 8 * r + p // 16] = 1.0
    ones64 = ones64.reshape(128, 8 * 64).astype(BF)
    # S2[p=(k8,i), r, m] = 1 iff m == 8*r + p//16   (A2 reduce, chunks 16/17)
    S2 = np.zeros((128, 2, 16), dtype=np.float32)
    for p in range(128):
        for r in range(2):
            S2[p, r, 8 * r + p // 16] = 1.0
    S2 = S2.reshape(128, 32).astype(BF)
    # S16[k16, r, p'=(k8l,i)] = 1 iff k16 == 8*r + p'//16 (c2 partition-bcast)
    S16 = np.zeros((16, 2, 128), dtype=np.float32)
    for k16 in range(16):
        for r in range(2):
            for pp in range(128):
                if k16 == 8 * r + pp // 16:
                    S16[k16, r, pp] = 1.0
    S16 = S16.reshape(16, 256).astype(BF)

    ins = []
    for c in range(NC):
        ks = c * KL
        Wc = Wf[:, ks:ks + KL]                 # [J, KL, O, I]
        uc = u[:, ks:ks + KL]                  # [B, KL, I]

        # w_s0 [128, (chunk, j, o)]: chunk rows (k8, i), free (j8, j4, o)
        ws0 = Wc.transpose(1, 3, 0, 2).reshape(KI, JO)       # [(k i), (j o)]
        ws0 = ws0.reshape(NCH, 128, JO).transpose(1, 0, 2).reshape(128, NCH * JO)
        # wo [128=(j4,o), (j8, k, i)], j = 4*j8 + j4
        wo = Wc.reshape(J8, J4, KL, O, I).transpose(1, 3, 0, 2, 4)
        wo = wo.reshape(J4 * O, J8 * KL * I)
        # w1a [128=k(0..127), (i, j, o)]
        w1a = Wc[:, :128].transpose(1, 3, 0, 2).reshape(128, I * JO)
        # urep3 [128, (chunk, j4, b)]: u[(k,i)] replicated over j4
        ur = uc.transpose(1, 2, 0).reshape(KI, 1, B)         # [(k i), 1, b]
        ur = np.broadcast_to(ur, (KI, J4, B)).reshape(KI, M128)
        ur = ur.reshape(NCH, 128, M128).transpose(1, 0, 2).reshape(128, NCH * M128)
        # u1a4 [128=k(0..127), (i, j4, b)]
        u4 = uc[:, :128].transpose(1, 2, 0).reshape(128, I, 1, B)
        u4 = np.broadcast_to(u4, (128, I, J4, B)).reshape(128, I * M128)

        ins.append({
            "w_s0": np.ascontiguousarray(ws0).astype(BF),
            "wo": np.ascontiguousarray(wo).astype(BF),
            "w1a": np.ascontiguousarray(w1a).astype(BF),
            "urep3": np.ascontiguousarray(ur).astype(BF),
            "u1a4": np.ascontiguousarray(u4).astype(BF),
            "ones64": ones64,
            "S2": S2,
            "S16": S16,
            "bias1": bias1,
        })
    return ins


def host_unpack(out):
    """out [(j4,b), (j8,o)] f32 -> [B, J, O] with j = 4*j8 + j4."""
    return np.ascontiguousarray(
        out.reshape(J4, B, J8, O).transpose(1, 2, 0, 3).reshape(B, J, O)
    )


# ---------------------------------------------------------------------------
# device program
# ---------------------------------------------------------------------------

def build_program(tc, outs, ins, n_cores=NC, use_cc=True, stop_after=None):
    import concourse.bass as bass  # noqa: F401
    from concourse import mybir, masks
    from concourse.tile import add_dep_helper

    F32 = mybir.dt.float32
    BF16 = mybir.dt.bfloat16
    ADD = mybir.AluOpType.add
    MULT = mybir.AluOpType.mult
    AX = mybir.AxisListType.X
    ACT = mybir.ActivationFunctionType

    nc = tc.nc
    ws0_d = ins["w_s0"]; wo_d = ins["wo"]; w1a_d = ins["w1a"]
    urep3_d = ins["urep3"]; u1a4_d = ins["u1a4"]
    ones64_d = ins["ones64"]; S2_d = ins["S2"]; S16_d = ins["S16"]
    bias1_d = ins["bias1"]
    out_d = outs["out"]

    import contextlib
    stack = contextlib.ExitStack()
    with stack:
        pool = stack.enter_context(tc.tile_pool(name="main", bufs=1))
        big = stack.enter_context(tc.tile_pool(name="big", bufs=1))
        psum = stack.enter_context(tc.tile_pool(name="psum", bufs=1, space="PSUM"))
        dram = stack.enter_context(tc.tile_pool(name="dram", bufs=1, space="DRAM"))

        # ---- resident inputs (DMA issue order = priority order) -------------
        urep3 = pool.tile([128, NCH * M128], BF16)
        ws0 = big.tile([128, NCH * JO], BF16)
        wo = big.tile([128, J8 * KI], BF16)
        w1a = big.tile([128, I * JO], BF16)
        u1a4 = pool.tile([128, I * M128], BF16)
        ones64 = pool.tile([128, 8 * 64], BF16)
        S2 = pool.tile([128, 32], BF16)
        S16 = pool.tile([16, 256], BF16)
        bias1 = pool.tile([M128, J8 * O], F32)
        ident = pool.tile([128, 128], BF16)

        # Front loads: ONLY what s0 needs (~5.6MB). The big wo/w1a loads are
        # deferred onto the gpsimd queue after the AR0 triggers: the first
        # collective's ring traffic shares DMA engines with input loads, so
        # front-loading everything delays the barrier/AllReduce by ~40us.
        nc.sync.dma_start(urep3[:], urep3_d)
        ws0v = ws0[:].rearrange("p (c f) -> p c f", c=NCH)
        ws0dv = ws0_d.rearrange("p (c f) -> p c f", c=NCH)
        # j8-half0 of all chunks first (gates s0-half0), split for pipelining
        for (a, b) in ((0, 3), (3, 6), (6, 9), (9, 12), (12, 15), (15, NCH)):
            nc.sync.dma_start(ws0v[:, a:b, 0:512], ws0dv[:, a:b, 0:512])
        nc.sync.dma_start(ws0v[:, :, 512:1024], ws0dv[:, :, 512:1024])
        for tile_, dram_ in ((ones64, ones64_d), (S2, S2_d), (S16, S16_d),
                             (bias1, bias1_d), (u1a4, u1a4_d)):
            nc.sync.dma_start(tile_[:], dram_)
        masks.make_identity(nc, ident[:])
        wov = wo[:].rearrange("p (c f) -> p c f", c=4)
        wodv = wo_d.rearrange("p (c f) -> p c f", c=4)

        # ---- collective bounce buffers (bf16, j8-halves) --------------------
        rg = [list(range(n_cores))]

        if SPLIT_AR0:
            cc0i = [dram.tile([M128, 128], BF16, name=f"cc0i{h}") for h in range(2)]
            cc0o = [dram.tile([M128, 128], BF16, name=f"cc0o{h}") for h in range(2)]
        else:
            cc0ib = dram.tile([M128, 256], BF16, name="cc0ib")
            cc0ob = dram.tile([M128, 256], BF16, name="cc0ob")
        if SPLIT_AR1:
            cc1i = [dram.tile([M128, 128], BF16, name=f"cc1i{h}") for h in range(2)]
            cc1o = [dram.tile([M128, 128], BF16, name=f"cc1o{h}") for h in range(2)]
        else:
            cc1ib = dram.tile([M128, 256], BF16, name="cc1ib")
            cc1ob = dram.tile([M128, 256], BF16, name="cc1ob")


        def _finish(tile_ap, rows):
            """Timing-bisect helper: route a dependency on `tile_ap` to out."""
            z = pool.tile([M128, J8 * O], F32, tag="finz")
            nc.vector.memset(z[:], 0.0)
            nc.vector.tensor_copy(z[:rows, :1], tile_ap[:rows, :1])
            nc.scalar.dma_start(out_d, z[:])

        # ---- s0 = sum_{k,i} u*W, chunked, j8-halves -------------------------
        s0c = [pool.tile([M128, 128], BF16, name=f"s0c{h}") for h in range(2)]
        for h in range(2):
            ps0 = psum.tile([M128, 512], F32, tag="acc", bufs=2, name=f"ps0_{h}")
            for cc in range(NCH):
                nc.tensor.matmul(
                    ps0[:], urep3[:, 128 * cc:128 * cc + 128],
                    ws0v[:, cc, 512 * h:512 * h + 512],
                    start=(cc == 0), stop=(cc == NCH - 1))
            # diagonal pick: rows (j4, b) want cols (j8, j4'=j4, o).
            # Split across Vector+Scalar to shorten the AR trigger path.
            pv = ps0[:].rearrange("m (j8 j4 o) -> m j8 j4 o", j8=4, j4=J4)
            dv = s0c[h][:].rearrange("m (j8 o) -> m j8 o", j8=4)
            for j4 in range(J4):
                if j4 % 2 == 0:
                    nc.vector.tensor_copy(
                        dv[32 * j4:32 * j4 + 32], pv[32 * j4:32 * j4 + 32, :, j4])
                else:
                    nc.scalar.activation(
                        dv[32 * j4:32 * j4 + 32], pv[32 * j4:32 * j4 + 32, :, j4],
                        ACT.Copy)
            if use_cc:
                if SPLIT_AR0:
                    bnc = nc.scalar.dma_start(cc0i[h][:], s0c[h][:])
                    nc.gpsimd.collective_compute(
                        "AllReduce", ADD, replica_groups=rg,
                        ins=[cc0i[h].opt()], outs=[cc0o[h].opt()])
                else:
                    bnc = nc.scalar.dma_start(
                        cc0ib[:, 128 * h:128 * h + 128], s0c[h][:])
                    if h == 1:
                        nc.gpsimd.collective_compute(
                            "AllReduce", ADD, replica_groups=rg,
                            ins=[cc0ib.opt()], outs=[cc0ob.opt()])
                if h == 0:
                    # wo load dispatches only once s0-half0's bounce is away:
                    # keeps the DMA engines clear of the CC barrier window.
                    for ch in range(4):
                        dins = nc.sync.dma_start(wov[:, ch], wodv[:, ch])
                        add_dep_helper(dins.ins, bnc.ins,
                                       reason="defer wo past AR0a bounce")
        if not use_cc:
            for ch in range(4):
                nc.sync.dma_start(wov[:, ch], wodv[:, ch])

        if stop_after == "s0":
            return _finish(s0c[1][:], M128)

        # ---- squash helper --------------------------------------------------
        epsb = pool.tile([128, 1], F32)
        nc.vector.memset(epsb[:], EPS)

        def squash_m(src, nj, tag):
            """m[128, nj]: per-(row, j) squash scale factor of src [128, (j, o)]."""
            t = pool.tile([M128, nj * O], F32, tag=f"sq_t{tag}", bufs=2)
            nc.vector.tensor_mul(t[:], src, src)
            sq = pool.tile([M128, nj], F32, tag=f"sq_s{tag}", bufs=2)
            nc.vector.tensor_reduce(
                sq[:], t[:].rearrange("p (j o) -> p j o", o=O), axis=AX, op=ADD)
            one = pool.tile([M128, nj], F32, tag=f"sq_o{tag}", bufs=2)
            nc.vector.tensor_scalar_add(one[:], sq[:], 1.0)
            sqr = pool.tile([M128, nj], F32, tag=f"sq_r{tag}", bufs=2)
            nc.scalar.activation(sqr[:], sq[:], ACT.Sqrt, bias=epsb[:M128])
            den = pool.tile([M128, nj], F32, tag=f"sq_d{tag}", bufs=2)
            nc.vector.tensor_mul(den[:], one[:], sqr[:])
            r = pool.tile([M128, nj], F32, tag=f"sq_rr{tag}", bufs=2)
            nc.vector.reciprocal(r[:], den[:])
            m = pool.tile([M128, nj], F32, tag=f"sq_m{tag}", bufs=2)
            nc.vector.tensor_mul(m[:], sq[:], r[:])
            return m

        def squash(dst, src, nj, tag):
            """dst[128, nj*O] = squash over o of src (same layout [(., j), o])."""
            m = squash_m(src, nj, tag)
            mv = m[:].unsqueeze(2).broadcast_to((M128, nj, O))
            nc.vector.tensor_mul(
                dst.rearrange("p (j o) -> p j o", o=O),
                src.rearrange("p (j o) -> p j o", o=O), mv)

        # ---- v0 halves: squash(s0/32 + bias) + transposes to v0bd -----------
        v0bd = pool.tile([128, J8 * 128], BF16)
        nc.vector.memset(v0bd[:], 0.0)
        v0bdv = v0bd[:].rearrange("p (j8 m) -> p j8 m", j8=J8)
        b1v = bias1[:].rearrange("m (j8 o) -> m j8 o", j8=J8)
        for h in range(2):
            if use_cc:
                s0g = pool.tile([M128, 128], BF16, tag="s0g", bufs=2)
                if SPLIT_AR0:
                    nc.scalar.dma_start(s0g[:], cc0o[h][:])
                else:
                    nc.scalar.dma_start(s0g[:], cc0ob[:, 128 * h:128 * h + 128])
            else:
                s0g = s0c[h]
            s0f = pool.tile([M128, 128], F32, tag="s0f", bufs=2)
            nc.vector.scalar_tensor_tensor(
                s0f[:], s0g[:], 1.0 / 32.0,
                b1v[:, 4 * h:4 * h + 4].rearrange("m j o -> m (j o)"),
                op0=MULT, op1=ADD)
            m = squash_m(s0f[:], 4, tag="v0")
            s0fv = s0f[:].rearrange("p (j o) -> p j o", o=O)
            # per-j8 final scale + transpose, so Wv can start on early j8s
            for jj in range(4):
                j8 = 4 * h + jj
                v0j = pool.tile([M128, O], BF16, tag="v0j", bufs=4)
                nc.vector.tensor_mul(
                    v0j[:], s0fv[:, jj],
                    m[:, jj:jj + 1].broadcast_to((M128, O)))
                pt = psum.tile([128, 128], BF16, tag="tr", bufs=2)
                nc.tensor.matmul(pt[:32, :], v0j[:],
                                 ident[:], is_transpose=True)
                for j4 in range(J4):
                    nc.vector.tensor_copy(
                        v0bdv[32 * j4:32 * j4 + 32, j8, 32 * j4:32 * j4 + 32],
                        pt[0:32, 32 * j4:32 * j4 + 32])

        if stop_after == "v0":
            return _finish(v0bd[:], 128)

        # ---- Wv chunks + fused u-mul + PE ones-reduce -> A ------------------
        # A psum tiles: per j8-half x; rows = k (0..127) / k-128 for A2
        wochv = wo[:].rearrange("p (j8 c f) -> p j8 c f", j8=J8, c=NCH)
        o64v = ones64[:].rearrange("p (r m) -> p r m", r=8)
        S2v = S2[:].rearrange("p (r m) -> p r m", r=2)
        GRP = [(0, 4), (4, 4), (8, 4), (12, 4), (16, 2)]
        A = pool.tile([128, FJ], F32)
        A2 = pool.tile([16, FJ], F32)
        e1 = pool.tile([128, J8 * B], F32)
        e1t = pool.tile([16, J8 * B], F32)
        for x in range(2):
            Aps = psum.tile([128, 512], F32, tag="Aps", name=f"Aps{x}")
            A2ps = psum.tile([16, 512], F32, tag="A2ps", name=f"A2ps{x}")
            for jj in range(4):
                j8 = 4 * x + jj
                for (c0, ng) in GRP:
                    pw = psum.tile([128, 512], F32, tag="wv", bufs=2)
                    for g in range(ng):
                        cc = c0 + g
                        nc.tensor.matmul(
                            pw[:, 128 * g:128 * g + 128],
                            wochv[:, j8, cc], v0bdv[:, j8],
                            start=True, stop=True)
                    mb = pool.tile([128, 512], BF16, tag="mb", bufs=3)
                    act_i = nc.scalar.activation(mb[:, :128 * ng],
                                                 pw[:, :128 * ng], ACT.Copy)
                    if x == 0 and jj == 2 and c0 == 0:
                        # w1a (s1 weights) load dispatches once Wv is running:
                        # fills the post-AR0 DMA-idle window, lands before s1.
                        dins = nc.sync.dma_start(w1a[:], w1a_d)
                        add_dep_helper(dins.ins, act_i.ins,
                                       reason="defer w1a into Wv phase")
                    nc.vector.tensor_mul(
                        mb[:, :128 * ng], mb[:, :128 * ng],
                        urep3[:, 128 * c0:128 * (c0 + ng)])
                    for g in range(ng):
                        cc = c0 + g
                        if cc < 16:
                            q = cc // 8
                            r = cc % 8
                            nc.tensor.matmul(
                                Aps[64 * q:64 * q + 64,
                                    128 * jj:128 * jj + 128],
                                o64v[:, r], mb[:, 128 * g:128 * g + 128],
                                start=(r == 0), stop=(r == 7))
                        else:
                            r = cc - 16
                            nc.tensor.matmul(
                                A2ps[:, 128 * jj:128 * jj + 128],
                                S2v[:, r], mb[:, 128 * g:128 * g + 128],
                                start=(r == 0), stop=(r == 1))
            # incremental softmax: exp + per-half partial sums overlap the
            # other half's Wv matmuls. |A| small => skip max subtraction.
            for (ps_, dst_, e_, P) in ((Aps, A, e1, 128), (A2ps, A2, e1t, 16)):
                h0 = 512 * x
                nc.scalar.activation(dst_[:P, h0:h0 + 512], ps_[:P], ACT.Exp)
                nc.vector.tensor_reduce(
                    e_[:P, 128 * x:128 * x + 128],
                    dst_[:P, h0:h0 + 512].rearrange(
                        "p (j8 j4 b) -> p j8 b j4", j8=4, j4=J4),
                    axis=AX, op=ADD)

        if stop_after == "A":
            return _finish(A[:], 128)

        # ---- softmax finalization: sum over j8, recip, normalize ------------
        c1 = pool.tile([128, FJ], BF16)
        c2 = pool.tile([16, FJ], BF16)

        def softmax_fin(src, dst, e_, P):
            e2 = pool.tile([128, B], F32, tag="sme2", bufs=2)
            nc.vector.tensor_reduce(
                e2[:P],
                e_[:P].rearrange("p (j8 b) -> p b j8", j8=J8),
                axis=AX, op=ADD)
            rr = pool.tile([128, B], F32, tag="smrr", bufs=2)
            nc.vector.reciprocal(rr[:P], e2[:P])
            rv = rr[:P].unsqueeze(1).unsqueeze(1).broadcast_to((P, J8, J4, B))
            nc.vector.tensor_mul(
                dst[:P].rearrange("p (j8 j4 b) -> p j8 j4 b", j8=J8, j4=J4),
                src[:P].rearrange("p (j8 j4 b) -> p j8 j4 b", j8=J8, j4=J4), rv)

        # A2 (16 rows, cheap) first: unlocks the cu-x PE broadcast, which then
        # runs while the big-A softmax finalizes on Vector.
        softmax_fin(A2, c2, e1t, 16)

        # ---- cu-x: chunks 16/17 stationaries via PE partition-bcast of c2 ---
        S16v = S16[:].rearrange("p (r m) -> p r m", r=2)
        cux = [pool.tile([128, J8 * 128], BF16, name=f"cux{r}") for r in range(2)]
        for r in range(2):
            for x in range(2):
                crp = psum.tile([128, 512], F32, tag="wv", bufs=2)
                nc.tensor.matmul(crp[:], S16v[:, r], c2[:, 512 * x:512 * x + 512],
                                 start=True, stop=True)
                nc.scalar.activation(cux[r][:, 512 * x:512 * x + 512],
                                     crp[:], ACT.Copy)
            urv = (urep3[:, 128 * (16 + r):128 * (16 + r) + 128]
                   .unsqueeze(1).broadcast_to((128, J8, 128)))
            nc.vector.tensor_mul(
                cux[r][:].rearrange("p (j8 m) -> p j8 m", j8=J8),
                cux[r][:].rearrange("p (j8 m) -> p j8 m", j8=J8), urv)
        cuxv = [cux[r][:].rearrange("p (j8 m) -> p j8 m", j8=J8) for r in range(2)]

        softmax_fin(A, c1, e1, 128)
        if stop_after == "c1":
            return _finish(c1[:], 128)

        # ---- per-j8: cu1 mul + s1 matmuls; j8-halves -> AllReduce -----------
        u14v = u1a4[:].rearrange("k (i m) -> k i m", i=I)
        c1v = c1[:].rearrange("k (j8 m) -> k j8 m", j8=J8)
        w1av = w1a[:].rearrange("k (i j8 m) -> k i j8 m", i=I, j8=J8)
        ws0cv = ws0[:].rearrange("p (c j8 m) -> p c j8 m", c=NCH, j8=J8)
        s1c = [pool.tile([M128, 128], BF16, name=f"s1c{h}") for h in range(2)]
        for h in range(2):
            sv = s1c[h][:].rearrange("m (j8 o) -> m j8 o", j8=4)
            for jj in range(4):
                j8 = 4 * h + jj
                cu1 = pool.tile([128, I * 128], BF16, tag="cu1", bufs=2)
                cu1v = cu1[:].rearrange("k (i m) -> k i m", i=I)
                cbc = c1v[:, j8].unsqueeze(1).broadcast_to((128, I, 128))
                nc.vector.tensor_mul(cu1v, u14v, cbc)
                ps1 = psum.tile([128, 128], F32, tag="acc", bufs=2)
                for i in range(I):
                    nc.tensor.matmul(ps1[:], cu1v[:, i], w1av[:, i, j8],
                                     start=(i == 0), stop=False)
                nc.tensor.matmul(ps1[:], cuxv[0][:, j8], ws0cv[:, 16, j8],
                                 start=False, stop=False)
                nc.tensor.matmul(ps1[:], cuxv[1][:, j8], ws0cv[:, 17, j8],
                                 start=False, stop=True)
                psv = ps1[:].rearrange("m (j4 o) -> m j4 o", j4=J4)
                for j4 in range(J4):
                    # scalar engine: vector is busy with cu1 muls here
                    nc.scalar.activation(
                        sv[32 * j4:32 * j4 + 32, jj],
                        psv[32 * j4:32 * j4 + 32, j4], ACT.Copy)
            if use_cc:
                if SPLIT_AR1:
                    nc.scalar.dma_start(cc1i[h][:], s1c[h][:])
                    nc.gpsimd.collective_compute(
                        "AllReduce", ADD, replica_groups=rg,
                        ins=[cc1i[h].opt()], outs=[cc1o[h].opt()])
                else:
                    nc.scalar.dma_start(
                        cc1ib[:, 128 * h:128 * h + 128], s1c[h][:])
                    if h == 1:
                        nc.gpsimd.collective_compute(
                            "AllReduce", ADD, replica_groups=rg,
                            ins=[cc1ib.opt()], outs=[cc1ob.opt()])

        if stop_after == "s1":
            return _finish(s1c[1][:], M128)

        # ---- out halves: squash(s1 + bias) ----------------------------------
        for h in range(2):
            if use_cc:
                s1g = pool.tile([M128, 128], BF16, tag="s1g", bufs=2)
                if SPLIT_AR1:
                    nc.scalar.dma_start(s1g[:], cc1o[h][:])
                else:
                    nc.scalar.dma_start(s1g[:], cc1ob[:, 128 * h:128 * h + 128])
            else:
                s1g = s1c[h]
            s1f = pool.tile([M128, 128], F32, tag="s1f", bufs=2)
            nc.vector.tensor_add(
                s1f[:], s1g[:],
                b1v[:, 4 * h:4 * h + 4].rearrange("m j o -> m (j o)"))
            v1 = pool.tile([M128, 128], F32, tag="v1", bufs=2)
            squash(v1[:], s1f[:], 4, tag="v1")
            nc.scalar.dma_start(out_d[:, 128 * h:128 * h + 128], v1[:])


# ---------------------------------------------------------------------------
# compile + run
# ---------------------------------------------------------------------------

_CACHE = {}


def _get_compiled(use_cc=True, n_cores=NC):
    key = (use_cc, n_cores)
    if key in _CACHE:
        return _CACHE[key]
    import concourse.bacc as bacc
    import concourse.tile as tile
    from concourse import mybir

    nc = bacc.Bacc("TRN2", target_bir_lowering=False, debug=False,
                   num_devices=n_cores)
    F32 = mybir.dt.float32
    BF16 = mybir.dt.bfloat16
    shapes = {
        "w_s0": ([128, NCH * JO], BF16),
        "wo": ([J4 * O, J8 * KI], BF16),
        "w1a": ([128, I * JO], BF16),
        "urep3": ([128, NCH * M128], BF16),
        "u1a4": ([128, I * M128], BF16),
        "ones64": ([128, 8 * 64], BF16),
        "S2": ([128, 32], BF16),
        "S16": ([16, 256], BF16),
        "bias1": ([M128, J8 * O], F32),
    }
    ins = {k: nc.dram_tensor(k, sh, dt, kind="ExternalInput").ap()
           for k, (sh, dt) in shapes.items()}
    outs = {"out": nc.dram_tensor("out", [M128, J8 * O], F32,
                                  kind="ExternalOutput").ap()}
    with tile.TileContext(nc) as tc:
        build_program(tc, outs, ins, n_cores=n_cores, use_cc=use_cc)
    nc.compile()
    _CACHE[key] = nc
    return nc


def kernel(**inputs):
    from concourse import bass_utils

    in_maps = host_prep(inputs["u"], inputs["W"], inputs["bias"])
    nc = _get_compiled()
    res = bass_utils.run_bass_kernel_spmd(nc, in_maps, core_ids=list(range(NC)))
    return host_unpack(np.asarray(res.results[0]["out"], dtype=np.float32))



# revision 15
# speedup vs baseline: 2.1149x; 1.0148x over previous
"""CapsNet dynamic-routing kernel for Trainium2, 8 NeuronCores.

Problem: nn_Caps_47742856462336
  u:    [32, 1152, 16] f32
  W:    [1, 32, 1152, 32, 16] f32
  bias: [1, 32, 32] f32
  out = 2-iter dynamic routing -> [32, 32, 32] f32

Sharding: tensor-parallel over in_caps (k): 1152/8 = 144 per core. Routing
state is combined with four small bf16 AllReduces (two j8-half pairs), each
overlapped with neighbouring compute. All cores end with identical output.

v2 design (all row spaces use (j4, b) = 128 partitions, j4-outer):
  The contraction index (k, i) = 144*16 = 2304 is split into 18 chunks of
  128 partitions (rows (k8, i), i inner, k = 8*chunk + k8).

  s0   = sum_{k,i} u*W        chunked PE: stationary urep3, moving w_s0
  v0   = squash(s0/32 + bias) after AllReduce (split in j8-halves)
  Wv   chunks: stationary wo-chunk [(j4 o), 128(ki)], moving v0bd ->
         psum [(ki), (j4 b)]; ACT-copy to bf16, DVE-mul by urep3 (2x mode)
  A    = sum_i (Wv*u): PE blockdiag ones-reduce over i -> A[k, (j8 j4 b)]
         (k on partitions => softmax over j is free-dim only, no max pass)
  c1   = exp(A) / sum_j exp(A)   (|A| <~ 20, no overflow)
  cu   = c1*u built directly on k-partitions (no transposes)
  s1   = per-j8 i-plane matmuls (k 0..127) + 2 chunked matmuls (k 128..143,
         via PE partition-broadcast of c2) -> AllReduce -> squash -> out
"""

import os
import sys
import numpy as np

for _p in ("/opt/trn_rl_repo", os.path.expanduser("~/.axon_site/_ro/trn_rl_repo")):
    if os.path.isdir(_p) and _p not in sys.path:
        sys.path.insert(0, _p)

import ml_dtypes  # noqa: E402

BF = ml_dtypes.bfloat16

B = 32      # batch
J = 32      # out_caps
O = 32      # out_dim
I = 16      # in_dim
KG = 1152   # global in_caps
NC = 8      # cores
KL = KG // NC   # 144 in_caps per core
KI = KL * I     # 2304 contraction size per core
NCH = KI // 128  # 18 chunks of 128 (k8, i) rows
EPS = 1e-7

J8 = 8   # j // 4
J4 = 4   # j %  4
JO = J * O           # 1024
M128 = J4 * B        # 128 rows (j4, b), j4-outer
FJ = J8 * M128       # 1024 free (j8, j4, b)

SPLIT_AR0 = True     # AllReduce s0 in two j8-halves (overlap 2nd with v0/Wv)
SPLIT_AR1 = True     # AllReduce s1 in two j8-halves (overlap with squash)


# ---------------------------------------------------------------------------
# host-side data prep: per-core DMA-friendly bf16/f32 layouts
# ---------------------------------------------------------------------------

def host_prep(u, W, bias):
    """Returns list of 8 dicts of named np arrays (the per-core DRAM inputs)."""
    u = np.asarray(u, dtype=np.float32)
    W = np.asarray(W, dtype=np.float32)
    bias = np.asarray(bias, dtype=np.float32)
    Wf = W[0]                      # [J, KG, O, I]
    biasf = bias[0]                # [J, O]

    # bias1[(j4 b), (j8 o)] = biasf[4*j8+j4, o]
    b1 = biasf.reshape(J8, J4, O).transpose(1, 0, 2)          # [j4, j8, o]
    b1 = np.broadcast_to(b1.reshape(J4, 1, J8 * O), (J4, B, J8 * O))
    bias1 = np.ascontiguousarray(b1.reshape(J4 * B, J8 * O), dtype=np.float32)

    # ones64[p=(k8,i), r, m=(c_sub, k8')] = 1 iff c_sub==r and k8'==p//16
    ones64 = np.zeros((128, 8, 64), dtype=np.float32)
    for p in range(128):
        for r in range(8):
            ones64[p, r, 8 * r + p // 16] = 1.0
    ones64 = ones64.reshape(128, 8 * 64).astype(BF)
    # S2[p=(k8,i), r, m] = 1 iff m == 8*r + p//16   (A2 reduce, chunks 16/17)
    S2 = np.zeros((128, 2, 16), dtype=np.float32)
    for p in range(128):
        for r in range(2):
            S2[p, r, 8 * r + p // 16] = 1.0
    S2 = S2.reshape(128, 32).astype(BF)
    # S16[k16, r, p'=(k8l,i)] = 1 iff k16 == 8*r + p'//16 (c2 partition-bcast)
    S16 = np.zeros((16, 2, 128), dtype=np.float32)
    for k16 in range(16):
        for r in range(2):
            for pp in range(128):
                if k16 == 8 * r + pp // 16:
                    S16[k16, r, pp] = 1.0
    S16 = S16.reshape(16, 256).astype(BF)

    ins = []
    for c in range(NC):
        ks = c * KL
        Wc = Wf[:, ks:ks + KL]                 # [J, KL, O, I]
        uc = u[:, ks:ks + KL]                  # [B, KL, I]

        # w_s0 [128, (chunk, j, o)]: chunk rows (k8, i), free (j8, j4, o)
        ws0 = Wc.transpose(1, 3, 0, 2).reshape(KI, JO)       # [(k i), (j o)]
        ws0 = ws0.reshape(NCH, 128, JO).transpose(1, 0, 2).reshape(128, NCH * JO)
        # wo [128=(j4,o), (j8, k, i)], j = 4*j8 + j4
        wo = Wc.reshape(J8, J4, KL, O, I).transpose(1, 3, 0, 2, 4)
        wo = wo.reshape(J4 * O, J8 * KL * I)
        # w1a [128=k(0..127), (i, j, o)]
        w1a = Wc[:, :128].transpose(1, 3, 0, 2).reshape(128, I * JO)
        # urep3 [128, (chunk, j4, b)]: u[(k,i)] replicated over j4
        ur = uc.transpose(1, 2, 0).reshape(KI, 1, B)         # [(k i), 1, b]
        ur = np.broadcast_to(ur, (KI, J4, B)).reshape(KI, M128)
        ur = ur.reshape(NCH, 128, M128).transpose(1, 0, 2).reshape(128, NCH * M128)
        # u1a4 [128=k(0..127), (i, j4, b)]
        u4 = uc[:, :128].transpose(1, 2, 0).reshape(128, I, 1, B)
        u4 = np.broadcast_to(u4, (128, I, J4, B)).reshape(128, I * M128)

        ins.append({
            "w_s0": np.ascontiguousarray(ws0).astype(BF),
            "wo": np.ascontiguousarray(wo).astype(BF),
            "w1a": np.ascontiguousarray(w1a).astype(BF),
            "urep3": np.ascontiguousarray(ur).astype(BF),
            "u1a4": np.ascontiguousarray(u4).astype(BF),
            "ones64": ones64,
            "S2": S2,
            "S16": S16,
            "bias1": bias1,
        })
    return ins


def host_unpack(out):
    """out [(j4,b), (j8,o)] f32 -> [B, J, O] with j = 4*j8 + j4."""
    return np.ascontiguousarray(
        out.reshape(J4, B, J8, O).transpose(1, 2, 0, 3).reshape(B, J, O)
    )


# ---------------------------------------------------------------------------
# device program
# ---------------------------------------------------------------------------

def build_program(tc, outs, ins, n_cores=NC, use_cc=True, stop_after=None):
    import concourse.bass as bass  # noqa: F401
    from concourse import mybir, masks
    from concourse.tile import add_dep_helper

    F32 = mybir.dt.float32
    BF16 = mybir.dt.bfloat16
    ADD = mybir.AluOpType.add
    MULT = mybir.AluOpType.mult
    AX = mybir.AxisListType.X
    ACT = mybir.ActivationFunctionType

    nc = tc.nc
    ws0_d = ins["w_s0"]; wo_d = ins["wo"]; w1a_d = ins["w1a"]
    urep3_d = ins["urep3"]; u1a4_d = ins["u1a4"]
    ones64_d = ins["ones64"]; S2_d = ins["S2"]; S16_d = ins["S16"]
    bias1_d = ins["bias1"]
    out_d = outs["out"]

    import contextlib
    stack = contextlib.ExitStack()
    with stack:
        pool = stack.enter_context(tc.tile_pool(name="main", bufs=1))
        big = stack.enter_context(tc.tile_pool(name="big", bufs=1))
        psum = stack.enter_context(tc.tile_pool(name="psum", bufs=1, space="PSUM"))
        dram = stack.enter_context(tc.tile_pool(name="dram", bufs=1, space="DRAM"))

        # ---- resident inputs (DMA issue order = priority order) -------------
        urep3 = pool.tile([128, NCH * M128], BF16)
        ws0 = big.tile([128, NCH * JO], BF16)
        wo = big.tile([128, J8 * KI], BF16)
        w1a = big.tile([128, I * JO], BF16)
        u1a4 = pool.tile([128, I * M128], BF16)
        ones64 = pool.tile([128, 8 * 64], BF16)
        S2 = pool.tile([128, 32], BF16)
        S16 = pool.tile([16, 256], BF16)
        bias1 = pool.tile([M128, J8 * O], F32)
        ident = pool.tile([128, 128], BF16)

        # Front loads: ONLY what s0 needs (~5.6MB). The big wo/w1a loads are
        # deferred onto the gpsimd queue after the AR0 triggers: the first
        # collective's ring traffic shares DMA engines with input loads, so
        # front-loading everything delays the barrier/AllReduce by ~40us.
        nc.sync.dma_start(urep3[:], urep3_d)
        ws0v = ws0[:].rearrange("p (c f) -> p c f", c=NCH)
        ws0dv = ws0_d.rearrange("p (c f) -> p c f", c=NCH)
        # j8-half0 of all chunks first (gates s0-half0), split for pipelining
        for (a, b) in ((0, 3), (3, 6), (6, 9), (9, 12), (12, 15), (15, NCH)):
            nc.sync.dma_start(ws0v[:, a:b, 0:512], ws0dv[:, a:b, 0:512])
        nc.sync.dma_start(ws0v[:, :, 512:1024], ws0dv[:, :, 512:1024])
        for tile_, dram_ in ((ones64, ones64_d), (S2, S2_d), (S16, S16_d),
                             (bias1, bias1_d), (u1a4, u1a4_d)):
            nc.sync.dma_start(tile_[:], dram_)
        masks.make_identity(nc, ident[:])
        wov = wo[:].rearrange("p (c f) -> p c f", c=4)
        wodv = wo_d.rearrange("p (c f) -> p c f", c=4)

        # ---- collective bounce buffers (bf16, j8-halves) --------------------
        rg = [list(range(n_cores))]

        if SPLIT_AR0:
            cc0i = [dram.tile([M128, 128], BF16, name=f"cc0i{h}") for h in range(2)]
            cc0o = [dram.tile([M128, 128], BF16, name=f"cc0o{h}",
                              addr_space="Shared") for h in range(2)]
        else:
            cc0ib = dram.tile([M128, 256], BF16, name="cc0ib")
            cc0ob = dram.tile([M128, 256], BF16, name="cc0ob")
        if SPLIT_AR1:
            cc1i = [dram.tile([M128, 128], BF16, name=f"cc1i{h}") for h in range(2)]
            cc1o = [dram.tile([M128, 128], BF16, name=f"cc1o{h}",
                              addr_space="Shared") for h in range(2)]
        else:
            cc1ib = dram.tile([M128, 256], BF16, name="cc1ib")
            cc1ob = dram.tile([M128, 256], BF16, name="cc1ob")


        def _finish(tile_ap, rows):
            """Timing-bisect helper: route a dependency on `tile_ap` to out."""
            z = pool.tile([M128, J8 * O], F32, tag="finz")
            nc.vector.memset(z[:], 0.0)
            nc.vector.tensor_copy(z[:rows, :1], tile_ap[:rows, :1])
            nc.scalar.dma_start(out_d, z[:])

        # ---- s0 = sum_{k,i} u*W, chunked, j8-halves -------------------------
        s0c = [pool.tile([M128, 128], BF16, name=f"s0c{h}") for h in range(2)]
        for h in range(2):
            ps0 = psum.tile([M128, 512], F32, tag="acc", bufs=2, name=f"ps0_{h}")
            for cc in range(NCH):
                nc.tensor.matmul(
                    ps0[:], urep3[:, 128 * cc:128 * cc + 128],
                    ws0v[:, cc, 512 * h:512 * h + 512],
                    start=(cc == 0), stop=(cc == NCH - 1))
            # diagonal pick: rows (j4, b) want cols (j8, j4'=j4, o).
            # Split across Vector+Scalar to shorten the AR trigger path.
            pv = ps0[:].rearrange("m (j8 j4 o) -> m j8 j4 o", j8=4, j4=J4)
            dv = s0c[h][:].rearrange("m (j8 o) -> m j8 o", j8=4)
            for j4 in range(J4):
                if j4 % 2 == 0:
                    nc.vector.tensor_copy(
                        dv[32 * j4:32 * j4 + 32], pv[32 * j4:32 * j4 + 32, :, j4])
                else:
                    nc.scalar.activation(
                        dv[32 * j4:32 * j4 + 32], pv[32 * j4:32 * j4 + 32, :, j4],
                        ACT.Copy)
            if use_cc:
                if SPLIT_AR0:
                    bnc = nc.scalar.dma_start(cc0i[h][:], s0c[h][:])
                    nc.gpsimd.collective_compute(
                        "AllReduce", ADD, replica_groups=rg,
                        ins=[cc0i[h].opt()], outs=[cc0o[h].opt()])
                else:
                    bnc = nc.scalar.dma_start(
                        cc0ib[:, 128 * h:128 * h + 128], s0c[h][:])
                    if h == 1:
                        nc.gpsimd.collective_compute(
                            "AllReduce", ADD, replica_groups=rg,
                            ins=[cc0ib.opt()], outs=[cc0ob.opt()])
                if h == 0:
                    # wo load dispatches only once s0-half0's bounce is away:
                    # keeps the DMA engines clear of the CC barrier window.
                    for ch in range(4):
                        dins = nc.sync.dma_start(wov[:, ch], wodv[:, ch])
                        add_dep_helper(dins.ins, bnc.ins,
                                       reason="defer wo past AR0a bounce")
        if not use_cc:
            for ch in range(4):
                nc.sync.dma_start(wov[:, ch], wodv[:, ch])

        if stop_after == "s0":
            return _finish(s0c[1][:], M128)

        # ---- squash helper --------------------------------------------------
        epsb = pool.tile([128, 1], F32)
        nc.vector.memset(epsb[:], EPS)

        def squash_m(src, nj, tag):
            """m[128, nj]: per-(row, j) squash scale factor of src [128, (j, o)]."""
            t = pool.tile([M128, nj * O], F32, tag=f"sq_t{tag}", bufs=2)
            nc.vector.tensor_mul(t[:], src, src)
            sq = pool.tile([M128, nj], F32, tag=f"sq_s{tag}", bufs=2)
            nc.vector.tensor_reduce(
                sq[:], t[:].rearrange("p (j o) -> p j o", o=O), axis=AX, op=ADD)
            one = pool.tile([M128, nj], F32, tag=f"sq_o{tag}", bufs=2)
            nc.vector.tensor_scalar_add(one[:], sq[:], 1.0)
            sqr = pool.tile([M128, nj], F32, tag=f"sq_r{tag}", bufs=2)
            nc.scalar.activation(sqr[:], sq[:], ACT.Sqrt, bias=epsb[:M128])
            den = pool.tile([M128, nj], F32, tag=f"sq_d{tag}", bufs=2)
            nc.vector.tensor_mul(den[:], one[:], sqr[:])
            r = pool.tile([M128, nj], F32, tag=f"sq_rr{tag}", bufs=2)
            nc.vector.reciprocal(r[:], den[:])
            m = pool.tile([M128, nj], F32, tag=f"sq_m{tag}", bufs=2)
            nc.vector.tensor_mul(m[:], sq[:], r[:])
            return m

        def squash(dst, src, nj, tag):
            """dst[128, nj*O] = squash over o of src (same layout [(., j), o])."""
            m = squash_m(src, nj, tag)
            mv = m[:].unsqueeze(2).broadcast_to((M128, nj, O))
            nc.vector.tensor_mul(
                dst.rearrange("p (j o) -> p j o", o=O),
                src.rearrange("p (j o) -> p j o", o=O), mv)

        # ---- v0 halves: squash(s0/32 + bias) + transposes to v0bd -----------
        v0bd = pool.tile([128, J8 * 128], BF16)
        nc.vector.memset(v0bd[:], 0.0)
        v0bdv = v0bd[:].rearrange("p (j8 m) -> p j8 m", j8=J8)
        b1v = bias1[:].rearrange("m (j8 o) -> m j8 o", j8=J8)
        for h in range(2):
            if use_cc:
                s0g = pool.tile([M128, 128], BF16, tag="s0g", bufs=2)
                if SPLIT_AR0:
                    nc.scalar.dma_start(s0g[:], cc0o[h][:])
                else:
                    nc.scalar.dma_start(s0g[:], cc0ob[:, 128 * h:128 * h + 128])
            else:
                s0g = s0c[h]
            s0f = pool.tile([M128, 128], F32, tag="s0f", bufs=2)
            nc.vector.scalar_tensor_tensor(
                s0f[:], s0g[:], 1.0 / 32.0,
                b1v[:, 4 * h:4 * h + 4].rearrange("m j o -> m (j o)"),
                op0=MULT, op1=ADD)
            m = squash_m(s0f[:], 4, tag="v0")
            s0fv = s0f[:].rearrange("p (j o) -> p j o", o=O)
            # per-j8 final scale + transpose, so Wv can start on early j8s
            for jj in range(4):
                j8 = 4 * h + jj
                v0j = pool.tile([M128, O], BF16, tag="v0j", bufs=4)
                nc.vector.tensor_mul(
                    v0j[:], s0fv[:, jj],
                    m[:, jj:jj + 1].broadcast_to((M128, O)))
                pt = psum.tile([128, 128], BF16, tag="tr", bufs=2)
                nc.tensor.matmul(pt[:32, :], v0j[:],
                                 ident[:], is_transpose=True)
                for j4 in range(J4):
                    nc.vector.tensor_copy(
                        v0bdv[32 * j4:32 * j4 + 32, j8, 32 * j4:32 * j4 + 32],
                        pt[0:32, 32 * j4:32 * j4 + 32])

        if stop_after == "v0":
            return _finish(v0bd[:], 128)

        # ---- Wv chunks + fused u-mul + PE ones-reduce -> A ------------------
        # A psum tiles: per j8-half x; rows = k (0..127) / k-128 for A2
        wochv = wo[:].rearrange("p (j8 c f) -> p j8 c f", j8=J8, c=NCH)
        o64v = ones64[:].rearrange("p (r m) -> p r m", r=8)
        S2v = S2[:].rearrange("p (r m) -> p r m", r=2)
        GRP = [(0, 4), (4, 4), (8, 4), (12, 4), (16, 2)]
        A = pool.tile([128, FJ], F32)
        A2 = pool.tile([16, FJ], F32)
        e1 = pool.tile([128, J8 * B], F32)
        e1t = pool.tile([16, J8 * B], F32)
        for x in range(2):
            Aps = psum.tile([128, 512], F32, tag="Aps", name=f"Aps{x}")
            A2ps = psum.tile([16, 512], F32, tag="A2ps", name=f"A2ps{x}")
            for jj in range(4):
                j8 = 4 * x + jj
                for (c0, ng) in GRP:
                    pw = psum.tile([128, 512], F32, tag="wv", bufs=2)
                    for g in range(ng):
                        cc = c0 + g
                        nc.tensor.matmul(
                            pw[:, 128 * g:128 * g + 128],
                            wochv[:, j8, cc], v0bdv[:, j8],
                            start=True, stop=True)
                    mb = pool.tile([128, 512], BF16, tag="mb", bufs=3)
                    act_i = nc.scalar.activation(mb[:, :128 * ng],
                                                 pw[:, :128 * ng], ACT.Copy)
                    if x == 0 and jj == 2 and c0 == 0:
                        # w1a (s1 weights) load dispatches once Wv is running:
                        # fills the post-AR0 DMA-idle window, lands before s1.
                        dins = nc.sync.dma_start(w1a[:], w1a_d)
                        add_dep_helper(dins.ins, act_i.ins,
                                       reason="defer w1a into Wv phase")
                    nc.vector.tensor_mul(
                        mb[:, :128 * ng], mb[:, :128 * ng],
                        urep3[:, 128 * c0:128 * (c0 + ng)])
                    for g in range(ng):
                        cc = c0 + g
                        if cc < 16:
                            q = cc // 8
                            r = cc % 8
                            nc.tensor.matmul(
                                Aps[64 * q:64 * q + 64,
                                    128 * jj:128 * jj + 128],
                                o64v[:, r], mb[:, 128 * g:128 * g + 128],
                                start=(r == 0), stop=(r == 7))
                        else:
                            r = cc - 16
                            nc.tensor.matmul(
                                A2ps[:, 128 * jj:128 * jj + 128],
                                S2v[:, r], mb[:, 128 * g:128 * g + 128],
                                start=(r == 0), stop=(r == 1))
            # incremental softmax: exp + per-half partial sums overlap the
            # other half's Wv matmuls. |A| small => skip max subtraction.
            for (ps_, dst_, e_, P) in ((Aps, A, e1, 128), (A2ps, A2, e1t, 16)):
                h0 = 512 * x
                nc.scalar.activation(dst_[:P, h0:h0 + 512], ps_[:P], ACT.Exp)
                nc.vector.tensor_reduce(
                    e_[:P, 128 * x:128 * x + 128],
                    dst_[:P, h0:h0 + 512].rearrange(
                        "p (j8 j4 b) -> p j8 b j4", j8=4, j4=J4),
                    axis=AX, op=ADD)

        if stop_after == "A":
            return _finish(A[:], 128)

        # ---- softmax finalization: sum over j8, recip, normalize ------------
        c1 = pool.tile([128, FJ], BF16)
        c2 = pool.tile([16, FJ], BF16)

        def softmax_fin(src, dst, e_, P):
            e2 = pool.tile([128, B], F32, tag="sme2", bufs=2)
            nc.vector.tensor_reduce(
                e2[:P],
                e_[:P].rearrange("p (j8 b) -> p b j8", j8=J8),
                axis=AX, op=ADD)
            rr = pool.tile([128, B], F32, tag="smrr", bufs=2)
            nc.vector.reciprocal(rr[:P], e2[:P])
            rv = rr[:P].unsqueeze(1).unsqueeze(1).broadcast_to((P, J8, J4, B))
            nc.vector.tensor_mul(
                dst[:P].rearrange("p (j8 j4 b) -> p j8 j4 b", j8=J8, j4=J4),
                src[:P].rearrange("p (j8 j4 b) -> p j8 j4 b", j8=J8, j4=J4), rv)

        # A2 (16 rows, cheap) first: unlocks the cu-x PE broadcast, which then
        # runs while the big-A softmax finalizes on Vector.
        softmax_fin(A2, c2, e1t, 16)

        # ---- cu-x: chunks 16/17 stationaries via PE partition-bcast of c2 ---
        S16v = S16[:].rearrange("p (r m) -> p r m", r=2)
        cux = [pool.tile([128, J8 * 128], BF16, name=f"cux{r}") for r in range(2)]
        for r in range(2):
            for x in range(2):
                crp = psum.tile([128, 512], F32, tag="wv", bufs=2)
                nc.tensor.matmul(crp[:], S16v[:, r], c2[:, 512 * x:512 * x + 512],
                                 start=True, stop=True)
                nc.scalar.activation(cux[r][:, 512 * x:512 * x + 512],
                                     crp[:], ACT.Copy)
            urv = (urep3[:, 128 * (16 + r):128 * (16 + r) + 128]
                   .unsqueeze(1).broadcast_to((128, J8, 128)))
            nc.vector.tensor_mul(
                cux[r][:].rearrange("p (j8 m) -> p j8 m", j8=J8),
                cux[r][:].rearrange("p (j8 m) -> p j8 m", j8=J8), urv)
        cuxv = [cux[r][:].rearrange("p (j8 m) -> p j8 m", j8=J8) for r in range(2)]

        softmax_fin(A, c1, e1, 128)
        if stop_after == "c1":
            return _finish(c1[:], 128)

        # ---- per-j8: cu1 mul + s1 matmuls; j8-halves -> AllReduce -----------
        u14v = u1a4[:].rearrange("k (i m) -> k i m", i=I)
        c1v = c1[:].rearrange("k (j8 m) -> k j8 m", j8=J8)
        w1av = w1a[:].rearrange("k (i j8 m) -> k i j8 m", i=I, j8=J8)
        ws0cv = ws0[:].rearrange("p (c j8 m) -> p c j8 m", c=NCH, j8=J8)
        s1c = [pool.tile([M128, 128], BF16, name=f"s1c{h}") for h in range(2)]
        for h in range(2):
            sv = s1c[h][:].rearrange("m (j8 o) -> m j8 o", j8=4)
            for jj in range(4):
                j8 = 4 * h + jj
                cu1 = pool.tile([128, I * 128], BF16, tag="cu1", bufs=2)
                cu1v = cu1[:].rearrange("k (i m) -> k i m", i=I)
                cbc = c1v[:, j8].unsqueeze(1).broadcast_to((128, I, 128))
                nc.vector.tensor_mul(cu1v, u14v, cbc)
                ps1 = psum.tile([128, 128], F32, tag="acc", bufs=2)
                for i in range(I):
                    nc.tensor.matmul(ps1[:], cu1v[:, i], w1av[:, i, j8],
                                     start=(i == 0), stop=False)
                nc.tensor.matmul(ps1[:], cuxv[0][:, j8], ws0cv[:, 16, j8],
                                 start=False, stop=False)
                nc.tensor.matmul(ps1[:], cuxv[1][:, j8], ws0cv[:, 17, j8],
                                 start=False, stop=True)
                psv = ps1[:].rearrange("m (j4 o) -> m j4 o", j4=J4)
                for j4 in range(J4):
                    # scalar engine: vector is busy with cu1 muls here
                    nc.scalar.activation(
                        sv[32 * j4:32 * j4 + 32, jj],
                        psv[32 * j4:32 * j4 + 32, j4], ACT.Copy)
            if use_cc:
                if SPLIT_AR1:
                    nc.scalar.dma_start(cc1i[h][:], s1c[h][:])
                    nc.gpsimd.collective_compute(
                        "AllReduce", ADD, replica_groups=rg,
                        ins=[cc1i[h].opt()], outs=[cc1o[h].opt()])
                else:
                    nc.scalar.dma_start(
                        cc1ib[:, 128 * h:128 * h + 128], s1c[h][:])
                    if h == 1:
                        nc.gpsimd.collective_compute(
                            "AllReduce", ADD, replica_groups=rg,
                            ins=[cc1ib.opt()], outs=[cc1ob.opt()])

        if stop_after == "s1":
            return _finish(s1c[1][:], M128)

        # ---- out halves: squash(s1 + bias) ----------------------------------
        for h in range(2):
            if use_cc:
                s1g = pool.tile([M128, 128], BF16, tag="s1g", bufs=2)
                if SPLIT_AR1:
                    nc.scalar.dma_start(s1g[:], cc1o[h][:])
                else:
                    nc.scalar.dma_start(s1g[:], cc1ob[:, 128 * h:128 * h + 128])
            else:
                s1g = s1c[h]
            s1f = pool.tile([M128, 128], F32, tag="s1f", bufs=2)
            nc.vector.tensor_add(
                s1f[:], s1g[:],
                b1v[:, 4 * h:4 * h + 4].rearrange("m j o -> m (j o)"))
            v1 = pool.tile([M128, 128], F32, tag="v1", bufs=2)
            squash(v1[:], s1f[:], 4, tag="v1")
            nc.scalar.dma_start(out_d[:, 128 * h:128 * h + 128], v1[:])


# ---------------------------------------------------------------------------
# compile + run
# ---------------------------------------------------------------------------

_CACHE = {}


def _get_compiled(use_cc=True, n_cores=NC):
    key = (use_cc, n_cores)
    if key in _CACHE:
        return _CACHE[key]
    import concourse.bacc as bacc
    import concourse.tile as tile
    from concourse import mybir

    nc = bacc.Bacc("TRN2", target_bir_lowering=False, debug=False,
                   num_devices=n_cores)
    F32 = mybir.dt.float32
    BF16 = mybir.dt.bfloat16
    shapes = {
        "w_s0": ([128, NCH * JO], BF16),
        "wo": ([J4 * O, J8 * KI], BF16),
        "w1a": ([128, I * JO], BF16),
        "urep3": ([128, NCH * M128], BF16),
        "u1a4": ([128, I * M128], BF16),
        "ones64": ([128, 8 * 64], BF16),
        "S2": ([128, 32], BF16),
        "S16": ([16, 256], BF16),
        "bias1": ([M128, J8 * O], F32),
    }
    ins = {k: nc.dram_tensor(k, sh, dt, kind="ExternalInput").ap()
           for k, (sh, dt) in shapes.items()}
    outs = {"out": nc.dram_tensor("out", [M128, J8 * O], F32,
                                  kind="ExternalOutput").ap()}
    with tile.TileContext(nc) as tc:
        build_program(tc, outs, ins, n_cores=n_cores, use_cc=use_cc)
    nc.compile()
    _CACHE[key] = nc
    return nc


def kernel(**inputs):
    from concourse import bass_utils

    in_maps = host_prep(inputs["u"], inputs["W"], inputs["bias"])
    nc = _get_compiled()
    res = bass_utils.run_bass_kernel_spmd(nc, in_maps, core_ids=list(range(NC)))
    return host_unpack(np.asarray(res.results[0]["out"], dtype=np.float32))

